# revision 17
# baseline (speedup 1.0000x reference)
"""Trainium2 Bass kernel for the controlled-U (CU) gate application.

Math: the reference builds U = P0 (x) I (x) ... + P1 (x) Mexp (x) I ...
with dim=2, wires=12, index=(0,1), control_state=(1,). This factors as

    U = diag(I_2, Mexp) (x) I_1024        (4096 x 4096)

so U @ x is:
    out[0:2048]     = x[0:2048]                        (identity)
    out[2048:3072]  = c00 * x[2048:3072] + c01 * x[3072:4096]
    out[3072:4096]  = c10 * x[2048:3072] + c11 * x[3072:4096]

with [[c00, c01], [c10, c11]] = Mexp = expm(M - M^H), a 2x2 unitary
computed on host (it is a 2x2 matrix; eigendecomposition of the
Hermitian generator gives the exact exponential).

Device strategy (8 NeuronCores, SPMD, batch-column sharding):
  - each core gets a (4096, 128) column shard of x_re / x_im
  - top 2048 rows: DVE strided copies interleave re/im -> complex64 layout
  - bottom 2048 rows: TensorE matmuls with diagonal stationary matrices
    (coefficients are *data*, so one compiled NEFF serves any M), PSUM
    accumulation, ACT engine interleave-copies PSUM -> SBUF
  - output per core: (4096, 256) f32 = interleaved complex; host gathers
    column shards and reinterprets as complex64 (zero-copy view).
"""

import numpy as np

import concourse.bass as bass
import concourse.bacc as bacc
import concourse.mybir as mybir
from concourse.tile import TileContext
from concourse.bass_utils import run_bass_kernel_spmd

# Problem geometry (hardcoded per the task contract).
D = 4096           # state dimension 2**12
B = 1024           # batch
NCORES = 8
BC = B // NCORES   # 128 batch columns per core
P = 128            # SBUF partitions
F32 = mybir.dt.float32
F32R = mybir.dt.float32r

NDIAG = 12         # 12 diagonal coefficient matrices (see _coef_values)


def _build_nc() -> bass.Bass:
    """Build the per-core Bass/Tile program (identical on all 8 cores)."""
    # Bacc (not raw Bass): its compile() lowers multi-dependency sync waits
    # through event semaphores — raw Bass trips walrus's per-instruction
    # wait-slot limit ("Too many sync wait commands").
    nc = bacc.Bacc("TRN2")

    xr = nc.dram_tensor("xr", [D, BC], F32, kind="ExternalInput")
    xi = nc.dram_tensor("xi", [D, BC], F32, kind="ExternalInput")
    coef = nc.dram_tensor("coef", [P, NDIAG * P], F32, kind="ExternalInput")
    out = nc.dram_tensor("out", [D, 2 * BC], F32, kind="ExternalOutput")

    # Row-block views: row = b*128 + p  ->  (p, b, j)
    xr_top = xr[0 : D // 2, :].rearrange("(b p) j -> p b j", p=P)      # (128, 16, 128)
    xi_top = xi[0 : D // 2, :].rearrange("(b p) j -> p b j", p=P)
    out_top = out[0 : D // 2, :].rearrange("(b p) j -> p b j", p=P)    # (128, 16, 256)

    # Bottom half split into h=0 (rows 2048:3072) / h=1 (rows 3072:4096),
    # k = block within half.  Pairing (h=0, h=1) at equal k keeps both
    # operands of the 2x2 mix in one tile.
    xr_bot = xr[D // 2 :, :].rearrange("(h k p) j -> p h k j", h=2, k=8)   # (128,2,8,128)
    xi_bot = xi[D // 2 :, :].rearrange("(h k p) j -> p h k j", h=2, k=8)
    out_bot = out[D // 2 :, :].rearrange("(h k p) j -> p h k j", h=2, k=8)  # (128,2,8,256)

    with TileContext(nc) as tc:
        with (
            tc.tile_pool(name="const", bufs=1) as const_pool,
            tc.tile_pool(name="io", bufs=3) as io_pool,
            tc.tile_pool(name="psum", bufs=7, space="PSUM") as psum_pool,
            tc.tile_pool(name="psum_warm", bufs=1, space="PSUM") as warm_pool,
        ):
            coef_sb = const_pool.tile([P, NDIAG * P], F32)
            nc.sync.dma_start(coef_sb[:], coef[:])

            def cdiag(k: int):
                return coef_sb[:, k * P : (k + 1) * P]

            # PE warmup matmul: its only dependency is the coef DMA, so each
            # later matmul introduces at most one new sync dependency.
            warm_ps = warm_pool.tile([P, 2], F32, tag="warm")
            nc.tensor.matmul(warm_ps[:], cdiag(0), coef_sb[:, 0:2],
                             start=True, stop=True)

            # ---- top half: identity, just interleave re/im ----
            TOPG = 8  # blocks per group
            for g in range(16 // TOPG):
                bs = slice(g * TOPG, (g + 1) * TOPG)
                xr_g = io_pool.tile([P, TOPG, BC], F32, tag="xr_top")
                xi_g = io_pool.tile([P, TOPG, BC], F32, tag="xi_top")
                nc.sync.dma_start(xr_g[:], xr_top[:, bs, :])
                nc.sync.dma_start(xi_g[:], xi_top[:, bs, :])
                o_g = io_pool.tile([P, TOPG, 2 * BC], F32, tag="out_top")
                nc.vector.tensor_copy(o_g[:, :, 0 : 2 * BC : 2], xr_g[:])
                nc.vector.tensor_copy(o_g[:, :, 1 : 2 * BC : 2], xi_g[:])
                # stores go on the ACT HWDGE ring: HWDGE is FIFO per issuing
                # engine, so a store waiting on compute must not block loads
                # (which are issued on the sync/SP ring and never wait).
                nc.scalar.dma_start(out_top[:, bs, :], o_g[:])

            # ---- bottom half: 2x2 complex mix on TensorE ----
            BOTG = 4  # k-blocks per group
            for g in range(8 // BOTG):
                ks = slice(g * BOTG, (g + 1) * BOTG)
                # one tile per (tensor, half) so each tile has exactly one
                # DMA writer -> each matmul adds at most one new sync wait.
                xr1_t = io_pool.tile([P, BOTG, BC], F32, tag="xr1")
                xi1_t = io_pool.tile([P, BOTG, BC], F32, tag="xi1")
                xr2_t = io_pool.tile([P, BOTG, BC], F32, tag="xr2")
                xi2_t = io_pool.tile([P, BOTG, BC], F32, tag="xi2")
                nc.sync.dma_start(xr1_t[:], xr_bot[:, 0, ks, :])
                nc.sync.dma_start(xi1_t[:], xi_bot[:, 0, ks, :])
                nc.sync.dma_start(xr2_t[:], xr_bot[:, 1, ks, :])
                nc.sync.dma_start(xi2_t[:], xi_bot[:, 1, ks, :])

                r1 = xr1_t[:]   # (128, 4, 128) free=512
                i1 = xi1_t[:]
                r2 = xr2_t[:]
                i2 = xi2_t[:]

                # accumulation recipes: psum_quantity -> [(diag_idx, moving), ...]
                recipes = {
                    "o1re": [(0, r1), (1, i1), (3, r2), (4, i2)],
                    "o1im": [(2, r1), (0, i1), (5, r2), (3, i2)],
                    "o2re": [(6, r1), (7, i1), (9, r2), (10, i2)],
                    "o2im": [(8, r1), (6, i1), (11, r2), (9, i2)],
                }
                ps = {}
                for name, terms in recipes.items():
                    pt = psum_pool.tile([P, BOTG, BC], F32, tag="ps")
                    for t, (k, mv) in enumerate(terms):
                        nc.tensor.matmul(
                            pt[:], cdiag(k), mv,
                            start=(t == 0), stop=(t == len(terms) - 1),
                        )
                    ps[name] = pt

                o_g = io_pool.tile([P, 2, BOTG, 2 * BC], F32, tag="out_bot")
                nc.scalar.copy(o_g[:, 0, :, 0 : 2 * BC : 2], ps["o1re"][:])
                nc.scalar.copy(o_g[:, 0, :, 1 : 2 * BC : 2], ps["o1im"][:])
                nc.scalar.copy(o_g[:, 1, :, 0 : 2 * BC : 2], ps["o2re"][:])
                nc.scalar.copy(o_g[:, 1, :, 1 : 2 * BC : 2], ps["o2im"][:])
                for h in range(2):
                    nc.scalar.dma_start(out_bot[:, h, ks, :], o_g[:, h])

    nc.finalize()
    return nc


_NC_CACHE = None


def _get_nc() -> bass.Bass:
    global _NC_CACHE
    if _NC_CACHE is None:
        _NC_CACHE = _build_nc()
    return _NC_CACHE


def _coef_values(M_re: np.ndarray, M_im: np.ndarray) -> np.ndarray:
    """Host-side 2x2 expm of the anti-Hermitian generator -> 12 diag values."""
    M = M_re.astype(np.float64) + 1j * M_im.astype(np.float64)
    A = M - M.conj().T          # anti-Hermitian
    H = -1j * A                 # Hermitian
    w, V = np.linalg.eigh(H)
    Mexp = V @ np.diag(np.exp(1j * w)) @ V.conj().T   # expm(A), exact
    c00, c01 = Mexp[0, 0], Mexp[0, 1]
    c10, c11 = Mexp[1, 0], Mexp[1, 1]
    vals = [
        c00.real, -c00.imag, c00.imag,
        c01.real, -c01.imag, c01.imag,
        c10.real, -c10.imag, c10.imag,
        c11.real, -c11.imag, c11.imag,
    ]
    coef = np.zeros((P, NDIAG * P), dtype=np.float32)
    idx = np.arange(P)
    for k, v in enumerate(vals):
        coef[idx, k * P + idx] = np.float32(v)
    return coef


def kernel(M_re, M_im, x_re, x_im) -> np.ndarray:
    M_re = np.asarray(M_re, dtype=np.float32)
    M_im = np.asarray(M_im, dtype=np.float32)
    x_re = np.asarray(x_re, dtype=np.float32)
    x_im = np.asarray(x_im, dtype=np.float32)

    coef = _coef_values(M_re, M_im)

    in_maps = []
    for d in range(NCORES):
        cols = slice(d * BC, (d + 1) * BC)
        in_maps.append({
            "xr": np.ascontiguousarray(x_re[:, cols]),
            "xi": np.ascontiguousarray(x_im[:, cols]),
            "coef": coef,
        })

    nc = _get_nc()
    res = run_bass_kernel_spmd(nc, in_maps, core_ids=list(range(NCORES)))
    full = np.concatenate([r["out"] for r in res.results], axis=1)  # (4096, 2048) f32
    return full.view(np.complex64)  # (4096, 1024)


# revision 23
# speedup vs baseline: 1.0673x; 1.0673x over previous
"""Trainium2 Bass kernel for the controlled-U (CU) gate application.

Math: the reference builds U = P0 (x) I (x) ... + P1 (x) Mexp (x) I ...
with dim=2, wires=12, index=(0,1), control_state=(1,). This factors as

    U = diag(I_2, Mexp) (x) I_1024        (4096 x 4096)

so U @ x is:
    out[0:2048]     = x[0:2048]                        (identity)
    out[2048:3072]  = c00 * x[2048:3072] + c01 * x[3072:4096]
    out[3072:4096]  = c10 * x[2048:3072] + c11 * x[3072:4096]

with [[c00, c01], [c10, c11]] = Mexp = expm(M - M^H), a 2x2 unitary
computed on host (it is a 2x2 matrix; eigendecomposition of the
Hermitian generator gives the exact exponential).

Device strategy (8 NeuronCores, SPMD, batch-column sharding):
  - each core gets a (4096, 128) column shard of x_re / x_im
  - top 2048 rows: DVE strided copies interleave re/im -> complex64 layout
  - bottom 2048 rows: TensorE matmuls with diagonal stationary matrices
    (coefficients are *data*, so one compiled NEFF serves any M), PSUM
    accumulation, ACT engine interleave-copies PSUM -> SBUF
  - output per core: (4096, 256) f32 = interleaved complex; host gathers
    column shards and reinterprets as complex64 (zero-copy view).
"""

import numpy as np

import concourse.bass as bass
import concourse.bacc as bacc
import concourse.mybir as mybir
from concourse.tile import TileContext
from concourse.bass_utils import run_bass_kernel_spmd

# Problem geometry (hardcoded per the task contract).
D = 4096           # state dimension 2**12
B = 1024           # batch
NCORES = 8
BC = B // NCORES   # 128 batch columns per core
P = 128            # SBUF partitions
F32 = mybir.dt.float32
F32R = mybir.dt.float32r

NDIAG = 12         # 12 diagonal coefficient matrices (see _coef_values)
TP = 32            # PE sub-tile size for tile_position concurrency


def _build_nc() -> bass.Bass:
    """Build the per-core Bass/Tile program (identical on all 8 cores)."""
    # Bacc (not raw Bass): its compile() lowers multi-dependency sync waits
    # through event semaphores — raw Bass trips walrus's per-instruction
    # wait-slot limit ("Too many sync wait commands").
    nc = bacc.Bacc("TRN2")

    xr = nc.dram_tensor("xr", [D, BC], F32, kind="ExternalInput")
    xi = nc.dram_tensor("xi", [D, BC], F32, kind="ExternalInput")
    # coef[p, k*TP + (p % TP)] = value_k  ->  [32p:32p+32, k*TP:(k+1)*TP]
    # is value_k * I_32 for any partition group p.
    coef = nc.dram_tensor("coef", [P, NDIAG * TP], F32, kind="ExternalInput")
    out = nc.dram_tensor("out", [D, 2 * BC], F32, kind="ExternalOutput")

    # Row-block views: row = b*128 + p  ->  (p, b, j)
    xr_top = xr[0 : D // 2, :].rearrange("(b p) j -> p b j", p=P)      # (128, 16, 128)
    xi_top = xi[0 : D // 2, :].rearrange("(b p) j -> p b j", p=P)
    out_top = out[0 : D // 2, :].rearrange("(b p) j -> p b j", p=P)    # (128, 16, 256)

    # Bottom half split into h=0 (rows 2048:3072) / h=1 (rows 3072:4096),
    # k = block within half.  Pairing (h=0, h=1) at equal k keeps both
    # operands of the 2x2 mix in one tile.
    xr_bot = xr[D // 2 :, :].rearrange("(h k p) j -> p h k j", h=2, k=8)   # (128,2,8,128)
    xi_bot = xi[D // 2 :, :].rearrange("(h k p) j -> p h k j", h=2, k=8)
    out_bot = out[D // 2 :, :].rearrange("(h k p) j -> p h k j", h=2, k=8)  # (128,2,8,256)

    with TileContext(nc) as tc:
        with (
            tc.tile_pool(name="const", bufs=1) as const_pool,
            tc.tile_pool(name="io", bufs=3) as io_pool,
            tc.tile_pool(name="psum", bufs=7, space="PSUM") as psum_pool,
            tc.tile_pool(name="psum_warm", bufs=1, space="PSUM") as warm_pool,
        ):
            coef_sb = const_pool.tile([P, NDIAG * TP], F32)
            nc.sync.dma_start(coef_sb[:], coef[:])

            def cdiag(k: int, i: int):
                """value_k * I_32 stationary for PE sub-tile row group i."""
                return coef_sb[i * TP : (i + 1) * TP, k * TP : (k + 1) * TP]

            # PE warmup matmul: its only dependency is the coef DMA, so each
            # later matmul introduces at most one new sync dependency.
            warm_ps = warm_pool.tile([P, 2], F32, tag="warm")
            nc.tensor.matmul(warm_ps[:TP], cdiag(0, 0), coef_sb[:TP, 0:2],
                             start=True, stop=True, tile_position=(0, 0))

            # ---- top half: identity, just interleave re/im ----
            TOPG = 8  # blocks per group
            for g in range(16 // TOPG):
                bs = slice(g * TOPG, (g + 1) * TOPG)
                xr_g = io_pool.tile([P, TOPG, BC], F32, tag="xr_top")
                xi_g = io_pool.tile([P, TOPG, BC], F32, tag="xi_top")
                nc.sync.dma_start(xr_g[:], xr_top[:, bs, :])
                nc.sync.dma_start(xi_g[:], xi_top[:, bs, :])
                o_g = io_pool.tile([P, TOPG, 2 * BC], F32, tag="out_top")
                nc.vector.tensor_copy(o_g[:, :, 0 : 2 * BC : 2], xr_g[:])
                nc.vector.tensor_copy(o_g[:, :, 1 : 2 * BC : 2], xi_g[:])
                # stores go on the ACT HWDGE ring: HWDGE is FIFO per issuing
                # engine, so a store waiting on compute must not block loads
                # (which are issued on the sync/SP ring and never wait).
                nc.scalar.dma_start(out_top[:, bs, :], o_g[:])

            # ---- bottom half: 2x2 complex mix on TensorE ----
            BOTG = 4  # k-blocks per group
            for g in range(8 // BOTG):
                ks = slice(g * BOTG, (g + 1) * BOTG)
                # one tile per (tensor, half) so each tile has exactly one
                # DMA writer -> each matmul adds at most one new sync wait.
                xr1_t = io_pool.tile([P, BOTG, BC], F32, tag="xr1")
                xi1_t = io_pool.tile([P, BOTG, BC], F32, tag="xi1")
                xr2_t = io_pool.tile([P, BOTG, BC], F32, tag="xr2")
                xi2_t = io_pool.tile([P, BOTG, BC], F32, tag="xi2")
                nc.sync.dma_start(xr1_t[:], xr_bot[:, 0, ks, :])
                nc.sync.dma_start(xi1_t[:], xi_bot[:, 0, ks, :])
                nc.sync.dma_start(xr2_t[:], xr_bot[:, 1, ks, :])
                nc.sync.dma_start(xi2_t[:], xi_bot[:, 1, ks, :])

                r1 = xr1_t[:]   # (128, 4, 128) free=512
                i1 = xi1_t[:]
                r2 = xr2_t[:]
                i2 = xi2_t[:]

                # accumulation recipes: psum_quantity -> [(diag_idx, moving), ...]
                recipes = {
                    "o1re": [(0, r1), (1, i1), (3, r2), (4, i2)],
                    "o1im": [(2, r1), (0, i1), (5, r2), (3, i2)],
                    "o2re": [(6, r1), (7, i1), (9, r2), (10, i2)],
                    "o2im": [(8, r1), (6, i1), (11, r2), (9, i2)],
                }
                ps = {}
                for name, terms in recipes.items():
                    pt = psum_pool.tile([P, BOTG, BC], F32, tag="ps")
                    for t, (k, mv) in enumerate(terms):
                        # fp32 matmul runs at 4 cyc/row; split into four
                        # 32x32 diagonal sub-tiles at PE positions
                        # (32i, 32i) which execute concurrently.
                        for i in range(P // TP):
                            nc.tensor.matmul(
                                pt[i * TP : (i + 1) * TP], cdiag(k, i),
                                mv[i * TP : (i + 1) * TP],
                                start=(t == 0), stop=(t == len(terms) - 1),
                                tile_position=(i * TP, i * TP),
                                # 4 concurrent groups share each PSUM bank
                                skip_group_check=True,
                            )
                    ps[name] = pt

                o_g = io_pool.tile([P, 2, BOTG, 2 * BC], F32, tag="out_bot")
                nc.scalar.copy(o_g[:, 0, :, 0 : 2 * BC : 2], ps["o1re"][:])
                nc.scalar.copy(o_g[:, 0, :, 1 : 2 * BC : 2], ps["o1im"][:])
                nc.scalar.copy(o_g[:, 1, :, 0 : 2 * BC : 2], ps["o2re"][:])
                nc.scalar.copy(o_g[:, 1, :, 1 : 2 * BC : 2], ps["o2im"][:])
                for h in range(2):
                    nc.scalar.dma_start(out_bot[:, h, ks, :], o_g[:, h])

    nc.finalize()
    return nc


_NC_CACHE = None


def _get_nc() -> bass.Bass:
    global _NC_CACHE
    if _NC_CACHE is None:
        _NC_CACHE = _build_nc()
    return _NC_CACHE


def _coef_values(M_re: np.ndarray, M_im: np.ndarray) -> np.ndarray:
    """Host-side 2x2 expm of the anti-Hermitian generator -> 12 diag values."""
    M = M_re.astype(np.float64) + 1j * M_im.astype(np.float64)
    A = M - M.conj().T          # anti-Hermitian
    H = -1j * A                 # Hermitian
    w, V = np.linalg.eigh(H)
    Mexp = V @ np.diag(np.exp(1j * w)) @ V.conj().T   # expm(A), exact
    c00, c01 = Mexp[0, 0], Mexp[0, 1]
    c10, c11 = Mexp[1, 0], Mexp[1, 1]
    vals = [
        c00.real, -c00.imag, c00.imag,
        c01.real, -c01.imag, c01.imag,
        c10.real, -c10.imag, c10.imag,
        c11.real, -c11.imag, c11.imag,
    ]
    coef = np.zeros((P, NDIAG * TP), dtype=np.float32)
    idx = np.arange(P)
    for k, v in enumerate(vals):
        coef[idx, k * TP + (idx % TP)] = np.float32(v)
    return coef


def kernel(M_re, M_im, x_re, x_im) -> np.ndarray:
    M_re = np.asarray(M_re, dtype=np.float32)
    M_im = np.asarray(M_im, dtype=np.float32)
    x_re = np.asarray(x_re, dtype=np.float32)
    x_im = np.asarray(x_im, dtype=np.float32)

    coef = _coef_values(M_re, M_im)

    in_maps = []
    for d in range(NCORES):
        cols = slice(d * BC, (d + 1) * BC)
        in_maps.append({
            "xr": np.ascontiguousarray(x_re[:, cols]),
            "xi": np.ascontiguousarray(x_im[:, cols]),
            "coef": coef,
        })

    nc = _get_nc()
    res = run_bass_kernel_spmd(nc, in_maps, core_ids=list(range(NCORES)))
    full = np.concatenate([r["out"] for r in res.results], axis=1)  # (4096, 2048) f32
    return full.view(np.complex64)  # (4096, 1024)


# revision 27
# speedup vs baseline: 1.3363x; 1.2521x over previous
"""Trainium2 Bass kernel for the controlled-U (CU) gate application.

Math: the reference builds U = P0 (x) I (x) ... + P1 (x) Mexp (x) I ...
with dim=2, wires=12, index=(0,1), control_state=(1,). This factors as

    U = diag(I_2, Mexp) (x) I_1024        (4096 x 4096)

so U @ x is:
    out[0:2048]     = x[0:2048]                        (identity)
    out[2048:3072]  = c00 * x[2048:3072] + c01 * x[3072:4096]
    out[3072:4096]  = c10 * x[2048:3072] + c11 * x[3072:4096]

with [[c00, c01], [c10, c11]] = Mexp = expm(M - M^H), a 2x2 unitary
computed on host (it is a 2x2 matrix; eigendecomposition of the
Hermitian generator gives the exact exponential).

Device strategy (8 NeuronCores, SPMD, batch-column sharding):
  - each core gets a (4096, 128) column shard of x_re / x_im
  - top 2048 rows: DVE strided copies interleave re/im -> complex64 layout
  - bottom 2048 rows: TensorE matmuls with diagonal stationary matrices
    (coefficients are *data*, so one compiled NEFF serves any M), PSUM
    accumulation, ACT engine interleave-copies PSUM -> SBUF
  - output per core: (4096, 256) f32 = interleaved complex; host gathers
    column shards and reinterprets as complex64 (zero-copy view).
"""

import numpy as np

import concourse.bass as bass
import concourse.bacc as bacc
import concourse.mybir as mybir
from concourse.tile import TileContext
from concourse.bass_utils import run_bass_kernel_spmd

# Problem geometry (hardcoded per the task contract).
D = 4096           # state dimension 2**12
B = 1024           # batch
NCORES = 8
BC = B // NCORES   # 128 batch columns per core
P = 128            # SBUF partitions
F32 = mybir.dt.float32
F32R = mybir.dt.float32r

NDIAG = 12         # 12 diagonal coefficient matrices (see _coef_values)
TP = 32            # PE sub-tile size for tile_position concurrency


def _build_nc() -> bass.Bass:
    """Build the per-core Bass/Tile program (identical on all 8 cores)."""
    # Bacc (not raw Bass): its compile() lowers multi-dependency sync waits
    # through event semaphores — raw Bass trips walrus's per-instruction
    # wait-slot limit ("Too many sync wait commands").
    nc = bacc.Bacc("TRN2")

    xr = nc.dram_tensor("xr", [D, BC], F32, kind="ExternalInput")
    xi = nc.dram_tensor("xi", [D, BC], F32, kind="ExternalInput")
    # coef[p, k*TP + (p % TP)] = value_k  ->  [32p:32p+32, k*TP:(k+1)*TP]
    # is value_k * I_32 for any partition group p.
    coef = nc.dram_tensor("coef", [P, NDIAG * TP], F32, kind="ExternalInput")
    cvec = nc.dram_tensor("cvec", [P, NDIAG], F32, kind="ExternalInput")
    out = nc.dram_tensor("out", [D, 2 * BC], F32, kind="ExternalOutput")

    # Row-block views: row = b*128 + p  ->  (p, b, j)
    xr_top = xr[0 : D // 2, :].rearrange("(b p) j -> p b j", p=P)      # (128, 16, 128)
    xi_top = xi[0 : D // 2, :].rearrange("(b p) j -> p b j", p=P)
    out_top = out[0 : D // 2, :].rearrange("(b p) j -> p b j", p=P)    # (128, 16, 256)

    # Bottom half split into h=0 (rows 2048:3072) / h=1 (rows 3072:4096),
    # k = block within half.  Pairing (h=0, h=1) at equal k keeps both
    # operands of the 2x2 mix in one tile.
    xr_bot = xr[D // 2 :, :].rearrange("(h k p) j -> p h k j", h=2, k=8)   # (128,2,8,128)
    xi_bot = xi[D // 2 :, :].rearrange("(h k p) j -> p h k j", h=2, k=8)
    out_bot = out[D // 2 :, :].rearrange("(h k p) j -> p h k j", h=2, k=8)  # (128,2,8,256)

    # quantity -> 4 coefficient indices, one per input (xr1, xi1, xr2, xi2).
    # (h_out, parity) selects the destination rows / interleave phase.
    RECIPES = [
        ("o1re", 0, 0, (0, 1, 3, 4)),
        ("o1im", 0, 1, (2, 0, 5, 3)),
        ("o2re", 1, 0, (6, 7, 9, 10)),
        ("o2im", 1, 1, (8, 6, 11, 9)),
    ]

    with TileContext(nc) as tc:
        with (
            tc.tile_pool(name="const", bufs=1) as const_pool,
            tc.tile_pool(name="io", bufs=3) as io_pool,
            tc.tile_pool(name="scr", bufs=2) as scr_pool,
            tc.tile_pool(name="psum", bufs=7, space="PSUM") as psum_pool,
            tc.tile_pool(name="psum_warm", bufs=1, space="PSUM") as warm_pool,
        ):
            coef_sb = const_pool.tile([P, NDIAG * TP], F32)
            nc.sync.dma_start(coef_sb[:], coef[:])
            cvec_sb = const_pool.tile([P, NDIAG], F32)
            nc.sync.dma_start(cvec_sb[:], cvec[:])

            def cdiag(k: int, i: int):
                """value_k * I_32 stationary for PE sub-tile row group i."""
                return coef_sb[i * TP : (i + 1) * TP, k * TP : (k + 1) * TP]

            def cval(k: int):
                """value_k as a per-partition scalar operand for the DVE."""
                return cvec_sb[:, k : k + 1]

            # PE warmup matmul: its only dependency is the coef DMA, so each
            # later matmul introduces at most one new sync dependency.
            warm_ps = warm_pool.tile([P, 2], F32, tag="warm")
            nc.tensor.matmul(warm_ps[:TP], cdiag(0, 0), coef_sb[:TP, 0:2],
                             start=True, stop=True, tile_position=(0, 0))

            # ---- bottom half first: loads gate PE/DVE compute ----
            BOTG = 4   # k-blocks per group
            bot_in = []
            for g in range(8 // BOTG):
                ks = slice(g * BOTG, (g + 1) * BOTG)
                # one tile per (tensor, half) so each tile has exactly one
                # DMA writer -> each matmul adds at most one new sync wait.
                xr1_t = io_pool.tile([P, BOTG, BC], F32, tag="xr1")
                xi1_t = io_pool.tile([P, BOTG, BC], F32, tag="xi1")
                xr2_t = io_pool.tile([P, BOTG, BC], F32, tag="xr2")
                xi2_t = io_pool.tile([P, BOTG, BC], F32, tag="xi2")
                nc.sync.dma_start(xr1_t[:], xr_bot[:, 0, ks, :])
                nc.sync.dma_start(xi1_t[:], xi_bot[:, 0, ks, :])
                nc.sync.dma_start(xr2_t[:], xr_bot[:, 1, ks, :])
                nc.sync.dma_start(xi2_t[:], xi_bot[:, 1, ks, :])
                bot_in.append((xr1_t, xi1_t, xr2_t, xi2_t))

            def pe_mix(o_g, ins, bs: slice):
                """2x2 complex mix for blocks bs on the TensorEngine."""
                nb = bs.stop - bs.start
                movs = [t[:, bs, :] for t in ins]       # free = nb*128
                for name, h, par, cks in RECIPES:
                    pt = psum_pool.tile([P, nb, BC], F32, tag="ps")
                    for t, (k, mv) in enumerate(zip(cks, movs)):
                        # fp32 matmul runs at 4 cyc/moving-column; the four
                        # 32x32 diagonal sub-tiles at PE positions (32i, 32i)
                        # execute concurrently.
                        for i in range(P // TP):
                            nc.tensor.matmul(
                                pt[i * TP : (i + 1) * TP], cdiag(k, i),
                                mv[i * TP : (i + 1) * TP],
                                start=(t == 0), stop=(t == 3),
                                tile_position=(i * TP, i * TP),
                                skip_group_check=True,
                            )
                    # ACT sits next to PSUM: interleave-copy PSUM -> SBUF
                    nc.scalar.copy(o_g[:, h, bs, par : 2 * BC : 2], pt[:])

            def dve_mix(o_g, ins, bs: slice):
                """2x2 complex mix for blocks bs on the VectorEngine."""
                nb = bs.stop - bs.start
                r1, i1, r2, i2 = (t[:, bs, :] for t in ins)
                for name, h, par, (ka, kb, kc, kd) in RECIPES:
                    t_a = scr_pool.tile([P, nb, BC], F32, tag="ta")
                    t_b = scr_pool.tile([P, nb, BC], F32, tag="tb")
                    nc.vector.tensor_scalar_mul(t_a[:], r1, cval(ka))
                    nc.vector.tensor_scalar_mul(t_b[:], i1, cval(kb))
                    nc.vector.tensor_add(t_a[:], t_a[:], t_b[:])
                    nc.vector.tensor_scalar_mul(t_b[:], r2, cval(kc))
                    nc.vector.tensor_add(t_a[:], t_a[:], t_b[:])
                    nc.vector.tensor_scalar_mul(t_b[:], i2, cval(kd))
                    nc.vector.tensor_add(
                        o_g[:, h, bs, par : 2 * BC : 2], t_a[:], t_b[:])

            for g in range(8 // BOTG):
                ks = slice(g * BOTG, (g + 1) * BOTG)
                o_g = io_pool.tile([P, 2, BOTG, 2 * BC], F32, tag="out_bot")
                if g == 0:
                    pe_mix(o_g, bot_in[g], slice(0, BOTG))
                else:
                    # split: 1 block on PE, 3 on DVE, to keep both engines
                    # under the DMA-bound critical path.
                    pe_mix(o_g, bot_in[g], slice(0, 1))
                    dve_mix(o_g, bot_in[g], slice(1, BOTG))
                # stores go on the ACT HWDGE ring: HWDGE is FIFO per issuing
                # engine, so a store waiting on compute must not block loads
                # (which are issued on the sync/SP ring and never wait).
                for h in range(2):
                    nc.scalar.dma_start(out_bot[:, h, ks, :], o_g[:, h])

            # ---- top half: identity, just interleave re/im ----
            TOPG = 4  # blocks per group
            for g in range(16 // TOPG):
                bs = slice(g * TOPG, (g + 1) * TOPG)
                xr_g = io_pool.tile([P, TOPG, BC], F32, tag="xr_top")
                xi_g = io_pool.tile([P, TOPG, BC], F32, tag="xi_top")
                nc.sync.dma_start(xr_g[:], xr_top[:, bs, :])
                nc.sync.dma_start(xi_g[:], xi_top[:, bs, :])
                o_g = io_pool.tile([P, TOPG, 2 * BC], F32, tag="out_top")
                nc.vector.tensor_copy(o_g[:, :, 0 : 2 * BC : 2], xr_g[:])
                nc.vector.tensor_copy(o_g[:, :, 1 : 2 * BC : 2], xi_g[:])
                nc.scalar.dma_start(out_top[:, bs, :], o_g[:])

    nc.finalize()
    return nc


_NC_CACHE = None


def _get_nc() -> bass.Bass:
    global _NC_CACHE
    if _NC_CACHE is None:
        _NC_CACHE = _build_nc()
    return _NC_CACHE


def _coef_values(M_re: np.ndarray, M_im: np.ndarray) -> np.ndarray:
    """Host-side 2x2 expm of the anti-Hermitian generator -> 12 diag values."""
    M = M_re.astype(np.float64) + 1j * M_im.astype(np.float64)
    A = M - M.conj().T          # anti-Hermitian
    H = -1j * A                 # Hermitian
    w, V = np.linalg.eigh(H)
    Mexp = V @ np.diag(np.exp(1j * w)) @ V.conj().T   # expm(A), exact
    c00, c01 = Mexp[0, 0], Mexp[0, 1]
    c10, c11 = Mexp[1, 0], Mexp[1, 1]
    vals = [
        c00.real, -c00.imag, c00.imag,
        c01.real, -c01.imag, c01.imag,
        c10.real, -c10.imag, c10.imag,
        c11.real, -c11.imag, c11.imag,
    ]
    coef = np.zeros((P, NDIAG * TP), dtype=np.float32)
    idx = np.arange(P)
    for k, v in enumerate(vals):
        coef[idx, k * TP + (idx % TP)] = np.float32(v)
    cvec = np.tile(np.array(vals, dtype=np.float32), (P, 1))
    return coef, cvec


def kernel(M_re, M_im, x_re, x_im) -> np.ndarray:
    M_re = np.asarray(M_re, dtype=np.float32)
    M_im = np.asarray(M_im, dtype=np.float32)
    x_re = np.asarray(x_re, dtype=np.float32)
    x_im = np.asarray(x_im, dtype=np.float32)

    coef, cvec = _coef_values(M_re, M_im)

    in_maps = []
    for d in range(NCORES):
        cols = slice(d * BC, (d + 1) * BC)
        in_maps.append({
            "xr": np.ascontiguousarray(x_re[:, cols]),
            "xi": np.ascontiguousarray(x_im[:, cols]),
            "coef": coef,
            "cvec": cvec,
        })

    nc = _get_nc()
    res = run_bass_kernel_spmd(nc, in_maps, core_ids=list(range(NCORES)))
    full = np.concatenate([r["out"] for r in res.results], axis=1)  # (4096, 2048) f32
    return full.view(np.complex64)  # (4096, 1024)


# revision 29
# speedup vs baseline: 1.4410x; 1.0784x over previous
"""Trainium2 Bass kernel for the controlled-U (CU) gate application.

Math: the reference builds U = P0 (x) I (x) ... + P1 (x) Mexp (x) I ...
with dim=2, wires=12, index=(0,1), control_state=(1,). This factors as

    U = diag(I_2, Mexp) (x) I_1024        (4096 x 4096)

so U @ x is:
    out[0:2048]     = x[0:2048]                        (identity)
    out[2048:3072]  = c00 * x[2048:3072] + c01 * x[3072:4096]
    out[3072:4096]  = c10 * x[2048:3072] + c11 * x[3072:4096]

with [[c00, c01], [c10, c11]] = Mexp = expm(M - M^H), a 2x2 unitary
computed exactly on host (eigendecomposition of the 2x2 Hermitian
generator).

Device strategy (8 NeuronCores, SPMD, row sharding — all DMA runs are
full 4 KiB rows):
  - core d gets top rows [256d, 256d+256) (identity) plus the bottom
    pair rows [2048+128d, +128) and [3072+128d, +128) (the 2x2 mix);
    every core runs the identical program on 1/8 of the work.
  - top rows: DVE strided copies interleave re/im -> complex64 layout
  - bottom pair rows, column half 0: TensorE fp32 matmuls with 32x32
    diagonal stationary tiles at concurrent tile positions, PSUM
    accumulation, ACT interleave-copies PSUM -> SBUF
  - bottom pair rows, column half 1: DVE tensor_scalar/tensor_tensor
    MACs (coefficients as per-partition scalars), writing the
    interleaved layout directly
  - outputs per core: f32 rows of interleaved (re, im) pairs; the host
    reassembles the (4096, 2048) f32 buffer and reinterprets it as
    complex64 (zero-copy view).

All arithmetic is fp32 (exact vs the reference up to rounding, ~1e-7).
"""

import numpy as np

import concourse.bacc as bacc
import concourse.mybir as mybir
from concourse.tile import TileContext
from concourse.bass_utils import run_bass_kernel_spmd

# Problem geometry (hardcoded per the task contract).
D = 4096           # state dimension 2**12
B = 1024           # batch
NCORES = 8
P = 128            # SBUF partitions
TROWS = D // 2 // NCORES   # 256 top (identity) rows per core
PROWS = D // 4 // NCORES   # 128 bottom pair rows per core
F32 = mybir.dt.float32

NDIAG = 12         # 12 diagonal coefficient matrices (see _coef_values)
TP = 32            # PE sub-tile size for tile_position concurrency
CH = B // 2        # column half processed per compute engine

# quantity -> (out half, interleave parity, coefficient idx per input).
# inputs are (xr1, xi1, xr2, xi2); coefficients include baked-in signs.
RECIPES = [
    ("o1re", 0, 0, (0, 1, 3, 4)),
    ("o1im", 0, 1, (2, 0, 5, 3)),
    ("o2re", 1, 0, (6, 7, 9, 10)),
    ("o2im", 1, 1, (8, 6, 11, 9)),
]


def _build_nc() -> bacc.Bacc:
    """Build the per-core Bass/Tile program (identical on all 8 cores)."""
    # Bacc (not raw Bass): its compile() lowers multi-dependency sync waits
    # through event semaphores — raw Bass trips walrus's per-instruction
    # wait-slot limit ("Too many sync wait commands").
    nc = bacc.Bacc("TRN2")

    xr_t = nc.dram_tensor("xr_t", [TROWS, B], F32, kind="ExternalInput")
    xi_t = nc.dram_tensor("xi_t", [TROWS, B], F32, kind="ExternalInput")
    xr_b1 = nc.dram_tensor("xr_b1", [PROWS, B], F32, kind="ExternalInput")
    xi_b1 = nc.dram_tensor("xi_b1", [PROWS, B], F32, kind="ExternalInput")
    xr_b2 = nc.dram_tensor("xr_b2", [PROWS, B], F32, kind="ExternalInput")
    xi_b2 = nc.dram_tensor("xi_b2", [PROWS, B], F32, kind="ExternalInput")
    # coef[p, k*TP + (p % TP)] = value_k  ->  32x32 diagonal blocks.
    coef = nc.dram_tensor("coef", [P, NDIAG * TP], F32, kind="ExternalInput")
    cvec = nc.dram_tensor("cvec", [P, NDIAG], F32, kind="ExternalInput")

    out_t = nc.dram_tensor("out_t", [TROWS, 2 * B], F32, kind="ExternalOutput")
    out_b1 = nc.dram_tensor("out_b1", [PROWS, 2 * B], F32, kind="ExternalOutput")
    out_b2 = nc.dram_tensor("out_b2", [PROWS, 2 * B], F32, kind="ExternalOutput")

    with TileContext(nc) as tc:
        with (
            tc.tile_pool(name="const", bufs=1) as const_pool,
            tc.tile_pool(name="io", bufs=2) as io_pool,
            tc.tile_pool(name="scr", bufs=2) as scr_pool,
            tc.tile_pool(name="psum", bufs=7, space="PSUM") as psum_pool,
            tc.tile_pool(name="psum_warm", bufs=1, space="PSUM") as warm_pool,
        ):
            coef_sb = const_pool.tile([P, NDIAG * TP], F32)
            nc.sync.dma_start(coef_sb[:], coef[:])
            cvec_sb = const_pool.tile([P, NDIAG], F32)
            nc.sync.dma_start(cvec_sb[:], cvec[:])

            def cdiag(k: int, i: int):
                """value_k * I_32 stationary for PE sub-tile row group i."""
                return coef_sb[i * TP : (i + 1) * TP, k * TP : (k + 1) * TP]

            def cval(k: int):
                """value_k as a per-partition scalar operand for the DVE."""
                return cvec_sb[:, k : k + 1]

            # PE warmup matmul: its only dependency is the coef DMA, so each
            # later matmul introduces at most one new sync dependency.
            warm_ps = warm_pool.tile([P, 2], F32, tag="warm")
            nc.tensor.matmul(warm_ps[:TP], cdiag(0, 0), coef_sb[:TP, 0:2],
                             start=True, stop=True, tile_position=(0, 0))

            # ---- bottom pair rows first: these gate PE/DVE compute ----
            b_in = {}
            for name, src in (("r1", xr_b1), ("i1", xi_b1),
                              ("r2", xr_b2), ("i2", xi_b2)):
                t = io_pool.tile([P, B], F32, tag=name)
                nc.sync.dma_start(t[:], src[:])
                b_in[name] = t

            o_b1 = io_pool.tile([P, 2 * B], F32, tag="o_b1")
            o_b2 = io_pool.tile([P, 2 * B], F32, tag="o_b2")
            o_b = {0: o_b1, 1: o_b2}

            # column half 0 -> TensorEngine (+ACT copies), half 1 -> DVE.
            cs_pe = slice(0, CH)
            cs_dve = slice(CH, B)

            for name, h, par, cks in RECIPES:
                pt = psum_pool.tile([P, CH], F32, tag="ps")
                movs = [b_in[n][:, cs_pe] for n in ("r1", "i1", "r2", "i2")]
                for t, (k, mv) in enumerate(zip(cks, movs)):
                    # fp32 matmul costs 4 cyc/moving-column; the four 32x32
                    # diagonal sub-tiles at positions (32i, 32i) execute
                    # concurrently.
                    for i in range(P // TP):
                        nc.tensor.matmul(
                            pt[i * TP : (i + 1) * TP], cdiag(k, i),
                            mv[i * TP : (i + 1) * TP],
                            start=(t == 0), stop=(t == 3),
                            tile_position=(i * TP, i * TP),
                            skip_group_check=True,
                        )
                # ACT sits next to PSUM: interleave-copy PSUM -> SBUF
                nc.scalar.copy(o_b[h][:, par : 2 * CH : 2], pt[:])

            for name, h, par, (ka, kb, kc, kd) in RECIPES:
                r1 = b_in["r1"][:, cs_dve]
                i1 = b_in["i1"][:, cs_dve]
                r2 = b_in["r2"][:, cs_dve]
                i2 = b_in["i2"][:, cs_dve]
                t_a = scr_pool.tile([P, CH], F32, tag="ta")
                t_b = scr_pool.tile([P, CH], F32, tag="tb")
                nc.vector.tensor_scalar_mul(t_a[:], r1, cval(ka))
                nc.vector.tensor_scalar_mul(t_b[:], i1, cval(kb))
                nc.vector.tensor_add(t_a[:], t_a[:], t_b[:])
                nc.vector.tensor_scalar_mul(t_b[:], r2, cval(kc))
                nc.vector.tensor_add(t_a[:], t_a[:], t_b[:])
                nc.vector.tensor_scalar_mul(t_b[:], i2, cval(kd))
                nc.vector.tensor_add(
                    o_b[h][:, 2 * CH + par : 2 * B : 2], t_a[:], t_b[:])

            # stores go on the ACT HWDGE ring: HWDGE is FIFO per issuing
            # engine, so a store waiting on compute must not block loads
            # (which are issued on the sync/SP ring and never wait).
            for h, dst in ((0, out_b1), (1, out_b2)):
                for c in range(2):
                    nc.scalar.dma_start(dst[:, c * B : (c + 1) * B],
                                        o_b[h][:, c * B : (c + 1) * B])

            # ---- top rows: identity, just interleave re/im ----
            for b in range(TROWS // P):
                rs = slice(b * P, (b + 1) * P)
                xr_g = io_pool.tile([P, B], F32, tag="xr_top")
                xi_g = io_pool.tile([P, B], F32, tag="xi_top")
                nc.sync.dma_start(xr_g[:], xr_t[rs, :])
                nc.sync.dma_start(xi_g[:], xi_t[rs, :])
                o_g = io_pool.tile([P, 2 * B], F32, tag="out_top")
                nc.vector.tensor_copy(o_g[:, 0 : 2 * B : 2], xr_g[:])
                nc.vector.tensor_copy(o_g[:, 1 : 2 * B : 2], xi_g[:])
                for c in range(2):
                    nc.scalar.dma_start(out_t[rs, c * B : (c + 1) * B],
                                        o_g[:, c * B : (c + 1) * B])

    nc.finalize()
    return nc


_NC_CACHE = None


def _get_nc() -> bacc.Bacc:
    global _NC_CACHE
    if _NC_CACHE is None:
        _NC_CACHE = _build_nc()
    return _NC_CACHE


def _coef_values(M_re: np.ndarray, M_im: np.ndarray):
    """Host-side 2x2 expm of the anti-Hermitian generator -> coef arrays."""
    M = M_re.astype(np.float64) + 1j * M_im.astype(np.float64)
    A = M - M.conj().T          # anti-Hermitian
    H = -1j * A                 # Hermitian
    w, V = np.linalg.eigh(H)
    Mexp = V @ np.diag(np.exp(1j * w)) @ V.conj().T   # expm(A), exact
    c00, c01 = Mexp[0, 0], Mexp[0, 1]
    c10, c11 = Mexp[1, 0], Mexp[1, 1]
    vals = [
        c00.real, -c00.imag, c00.imag,
        c01.real, -c01.imag, c01.imag,
        c10.real, -c10.imag, c10.imag,
        c11.real, -c11.imag, c11.imag,
    ]
    coef = np.zeros((P, NDIAG * TP), dtype=np.float32)
    idx = np.arange(P)
    for k, v in enumerate(vals):
        coef[idx, k * TP + (idx % TP)] = np.float32(v)
    cvec = np.tile(np.array(vals, dtype=np.float32), (P, 1))
    return coef, cvec


def _in_map(x_re, x_im, coef, cvec, d: int) -> dict:
    t0 = d * TROWS
    b1 = D // 2 + d * PROWS
    b2 = 3 * D // 4 + d * PROWS
    return {
        "xr_t": x_re[t0 : t0 + TROWS],
        "xi_t": x_im[t0 : t0 + TROWS],
        "xr_b1": x_re[b1 : b1 + PROWS],
        "xi_b1": x_im[b1 : b1 + PROWS],
        "xr_b2": x_re[b2 : b2 + PROWS],
        "xi_b2": x_im[b2 : b2 + PROWS],
        "coef": coef,
        "cvec": cvec,
    }


def kernel(M_re, M_im, x_re, x_im) -> np.ndarray:
    M_re = np.asarray(M_re, dtype=np.float32)
    M_im = np.asarray(M_im, dtype=np.float32)
    x_re = np.ascontiguousarray(x_re, dtype=np.float32)
    x_im = np.ascontiguousarray(x_im, dtype=np.float32)

    coef, cvec = _coef_values(M_re, M_im)
    in_maps = [_in_map(x_re, x_im, coef, cvec, d) for d in range(NCORES)]

    nc = _get_nc()
    res = run_bass_kernel_spmd(nc, in_maps, core_ids=list(range(NCORES)))

    full = np.empty((D, 2 * B), dtype=np.float32)
    for d, r in enumerate(res.results):
        t0 = d * TROWS
        b1 = D // 2 + d * PROWS
        b2 = 3 * D // 4 + d * PROWS
        full[t0 : t0 + TROWS] = r["out_t"]
        full[b1 : b1 + PROWS] = r["out_b1"]
        full[b2 : b2 + PROWS] = r["out_b2"]
    return full.view(np.complex64)  # (4096, 1024)
